# revision 46
# baseline (speedup 1.0000x reference)
"""Fused pairwise-MLP kernel for Trainium2 (8 NeuronCores, SPMD data-parallel).

Computes log_q[i, j] = W3 @ gelu(W2 @ gelu(a[j] + b[i] + b1) + b2) + b3
with a = z1 @ W1a.T, b = z2 @ W1b.T  (W1 = [W1a | W1b]), N=1024, H=EMB=128.

Sharding: rows of i (z2) split across 8 cores, z1 + weights replicated
(host-side sharding; no collectives). The [N, N, H] intermediates are never
materialized in HBM — everything stays in SBUF/PSUM per 128-row i-tile.

The host only relays/relabels inputs (transposes, sharding, zero-padding W3);
all math runs on device. Per core and per i: one 1024-wide gelu on ACT
(bias port adds b[i]+b1), two fp32r W2 matmuls, a second gelu, and the W3
dot as bf16 M=32 column-tile matmuls batched 4-concurrent per quad of i's.
The kernel is ACT-bound: 2 gelu passes over 16.8M elements per core at
1 elem/lane/cycle @ 1.2 GHz.
"""

import numpy as np

import concourse.bacc as bacc
import concourse.bass as bass
import concourse.tile as tile
import concourse.mybir as mybir
from concourse import bass_utils



N = 1024
EMB = 128
HID = 128
NCORES = 8
SH = N // NCORES  # i-rows per core
F32 = mybir.dt.float32
F32R = mybir.dt.float32r  # fp32 bits, single-pass reduced-precision matmul
BF16 = mybir.dt.bfloat16
GELU = mybir.ActivationFunctionType.Gelu


def _build():
    nc = bacc.Bacc("TRN2", target_bir_lowering=False, debug=False)

    z1Tq_d = [
        nc.dram_tensor(f"z1Tq{q}", (EMB, 256), F32, kind="ExternalInput")
        for q in range(4)
    ]
    z2T_d = nc.dram_tensor("z2T", (EMB, SH), F32, kind="ExternalInput")
    w1aT_d = nc.dram_tensor("w1aT", (EMB, HID), F32, kind="ExternalInput")
    w1bT_d = nc.dram_tensor("w1bT", (EMB, HID), F32, kind="ExternalInput")
    w2T_d = nc.dram_tensor("w2T", (HID, HID), F32, kind="ExternalInput")
    w3p_d = nc.dram_tensor("w3p", (HID, 32), F32, kind="ExternalInput")
    b1_d = nc.dram_tensor("b1", (HID,), F32, kind="ExternalInput")
    b2_d = nc.dram_tensor("b2", (HID,), F32, kind="ExternalInput")
    b3_d = nc.dram_tensor("b3", (1,), F32, kind="ExternalInput")
    out_d = nc.dram_tensor("out", (SH, N), F32, kind="ExternalOutput")

    with tile.TileContext(nc) as tc:
        _body(
            tc, out_d, z1Tq_d, z2T_d, w1aT_d, w1bT_d, w2T_d, w3p_d,
            b1_d, b2_d, b3_d,
        )

    nc.compile()
    return nc


def _body(tc, out_d, z1Tq_d, z2T_d, w1aT_d, w1bT_d, w2T_d, w3p_d, b1_d, b2_d, b3_d):
    nc = tc.nc
    with (
        tc.tile_pool(name="const", bufs=1) as const,
        tc.tile_pool(name="h1p", bufs=4) as h1p,
        tc.tile_pool(name="h2p", bufs=8) as h2p,
        tc.tile_pool(name="srows", bufs=8) as srows,
        tc.tile_pool(name="ps", bufs=3, space="PSUM") as ps,
        tc.tile_pool(name="rowp", bufs=1, space="PSUM") as rowp,
    ):
        # ---- load inputs: z1T fans out across the sync+scalar HWDGE queues,
        # small tensors ride the gpsimd SWDGE queue ----
        w1aT_sb = const.tile([128, HID], F32)
        nc.sync.dma_start(out=w1aT_sb, in_=w1aT_d.ap())
        z1T_sb = const.tile([128, N], F32)
        for q, eng in enumerate((nc.sync, nc.scalar, nc.sync, nc.scalar)):
            eng.dma_start(out=z1T_sb[:, q * 256 : (q + 1) * 256], in_=z1Tq_d[q].ap())
        z2T_sb = const.tile([128, SH], F32)
        nc.sync.dma_start(out=z2T_sb, in_=z2T_d.ap())
        w1bT_sb = const.tile([128, HID], F32)
        nc.scalar.dma_start(out=w1bT_sb, in_=w1bT_d.ap())
        w2T_f = const.tile([128, HID], F32)
        nc.scalar.dma_start(out=w2T_f, in_=w2T_d.ap())
        w3p_f = const.tile([128, 32], F32)
        nc.gpsimd.dma_start(out=w3p_f, in_=w3p_d.ap())
        b1_sb = const.tile([128, 1], F32)
        nc.sync.dma_start(out=b1_sb, in_=b1_d.ap().rearrange("(p o) -> p o", o=1))
        b2_sb = const.tile([128, 1], F32)
        nc.gpsimd.dma_start(out=b2_sb, in_=b2_d.ap().rearrange("(p o) -> p o", o=1))
        b3_sb = const.tile([128, 1], F32)
        nc.gpsimd.dma_start(
            out=b3_sb,
            in_=bass.AP(tensor=b3_d, offset=0, ap=[[0, 128], [1, 1]]),
        )

        # Dummy 1-element gelu so the ~2.7us ACT table load for the gelu set
        # runs right away, off the critical path of the first real gelu.
        tiny = const.tile([1, 1], F32)
        nc.vector.memset(tiny, 0.0)
        warm = const.tile([1, 1], F32)
        nc.scalar.activation(warm, tiny, GELU)

        # fp32r / bf16 lhsT casts (DVE rounds on write, as the fp32r matmul
        # consumer requires)
        w2T_sb = const.tile([128, HID], F32R)
        nc.vector.tensor_copy(w2T_sb, w2T_f)
        w3T_sb = const.tile([128, 32], BF16)
        nc.vector.tensor_copy(w3T_sb, w3p_f)

        # ---- a[h, j] for all j (SBUF: DVE pre-adds read it); b_pp = b + b1 ----
        tpa = ps.tile([128, 1024], F32, tag="ps1")
        for q in range(4):
            nc.tensor.matmul(
                tpa[:, q * 256 : (q + 1) * 256],
                w1aT_sb,
                z1T_sb[:, q * 256 : (q + 1) * 256],
            )
        tpb = ps.tile([128, 1024], F32, tag="ps1")
        nc.tensor.matmul(tpb[:, 0:SH], w1bT_sb, z2T_sb)
        b_pp_sb = const.tile([128, SH], F32)
        nc.vector.tensor_scalar_add(b_pp_sb, tpb[:, 0:SH], b1_sb[:, 0:1])

        # a stays in PSUM for the head quads' gelus; the SBUF copy (for the
        # steady-state DVE pre-adds) happens off the critical path.
        a_sb = const.tile([128, N], F32)
        nc.vector.tensor_copy(a_sb, tpa)

        # ---- main loop over my 128 i values, processed as 32 quads ----
        # DVE precomputes pre[:, k] = a + b_pp[:, i] for the 4 i's of a quad
        # (2x-mode SBUF adds) so ACT runs one bias-free 4096-wide gelu1 per
        # quad plus one 1024-wide gelu2 per i, emitted 2 quads ahead. The
        # W3-dot runs as 4 concurrent bf16 M=32 column-tile matmuls (outputs
        # at PSUM partitions 0/32/64/96); one strided DVE op evacuates all 4
        # rows, and one DMA per quad writes them out.
        NQ = SH // 4
        h1qs = [None] * NQ

        def emit_g1_quad(q):
            h1q = h1p.tile([128, 4 * N], F32R, tag="h1q", name="h1q", bufs=3)
            if q < 2:
                # Head quads: per-i gelu with the ACT bias port, streaming a
                # straight from PSUM — skips the DVE pre-add chain and the
                # a->SBUF copy on the kernel's critical path.
                for k in range(4):
                    i = 4 * q + k
                    nc.scalar.activation(
                        h1q[:, k * N : (k + 1) * N],
                        tpa,
                        GELU,
                        bias=b_pp_sb[:, i : i + 1],
                    )
            else:
                pre = h1p.tile([128, 4 * N], F32, tag="pre", name="pre", bufs=3)
                for k in range(4):
                    i = 4 * q + k
                    nc.vector.tensor_scalar_add(
                        pre[:, k * N : (k + 1) * N], a_sb, b_pp_sb[:, i : i + 1]
                    )
                nc.scalar.activation(h1q, pre, GELU)
            h1qs[q] = h1q

        emit_g1_quad(0)
        emit_g1_quad(1)
        h2s = [None] * 4
        for q in range(NQ):
            if q + 2 < NQ:
                emit_g1_quad(q + 2)
            h1q = h1qs[q]
            h1qs[q] = None

            for k in range(4):
                base = k * N
                ps1 = ps.tile([128, N], F32, tag="ps1")
                nc.tensor.matmul(ps1[:, 0:512], w2T_sb, h1q[:, base : base + 512])
                nc.tensor.matmul(
                    ps1[:, 512:1024], w2T_sb, h1q[:, base + 512 : base + 1024]
                )
                h2 = h2p.tile([128, N], BF16, tag="h2")
                nc.scalar.activation(h2, ps1, GELU, bias=b2_sb[:, 0:1])
                h2s[k] = h2

            pr = rowp.tile([128, N], F32, tag="row", name="pr")
            for k in range(4):
                nc.tensor.matmul(
                    pr[32 * k : 32 * k + 32, 0:512],
                    w3T_sb,
                    h2s[k][:, 0:512],
                    tile_position=(0, 32 * k),
                )
                nc.tensor.matmul(
                    pr[32 * k : 32 * k + 32, 512:1024],
                    w3T_sb,
                    h2s[k][:, 512:1024],
                    tile_position=(0, 32 * k),
                )
            # One full-width DVE op evacuates all 4 result rows (at partitions
            # 0/32/64/96; cost is free-dim-bound, same as a single row); the
            # DMA gathers the 4 strided rows out.
            srow4 = srows.tile([128, N], F32, tag="srow", bufs=3)
            nc.vector.tensor_scalar_add(srow4, pr, b3_sb[:, 0:1])
            nc.sync.dma_start(
                out=out_d.ap()[4 * q : 4 * q + 4, :], in_=srow4[0:128:32, :]
            )


_NC_CACHE = None


def make_in_maps(z1, z2, W1, b1, W2, b2, W3, b3):
    f = np.float32
    z1 = np.asarray(z1, dtype=f)
    z2 = np.asarray(z2, dtype=f)
    W1 = np.asarray(W1, dtype=f)
    b1 = np.ascontiguousarray(np.asarray(b1, dtype=f))
    W2 = np.asarray(W2, dtype=f)
    b2 = np.ascontiguousarray(np.asarray(b2, dtype=f))
    W3 = np.asarray(W3, dtype=f)
    b3 = np.ascontiguousarray(np.asarray(b3, dtype=f))

    # Host-side relayout only (no math): transposes, the i-shard split of
    # z2, and zero-padding W3 to an M=32 column tile.
    z1T = np.ascontiguousarray(z1.T)
    z1Tq = {
        f"z1Tq{q}": np.ascontiguousarray(z1T[:, q * 256 : (q + 1) * 256])
        for q in range(4)
    }
    w1aT = np.ascontiguousarray(W1[:, :EMB].T)
    w1bT = np.ascontiguousarray(W1[:, EMB:].T)
    w2T = np.ascontiguousarray(W2.T)
    w3p = np.zeros((HID, 32), dtype=f)
    w3p[:, 0] = W3[0]

    return [
        {
            **z1Tq,
            "z2T": np.ascontiguousarray(z2[c * SH : (c + 1) * SH].T),
            "w1aT": w1aT,
            "w1bT": w1bT,
            "w2T": w2T,
            "w3p": w3p,
            "b1": b1,
            "b2": b2,
            "b3": b3,
        }
        for c in range(NCORES)
    ]


def kernel(z1, z2, W1, b1, W2, b2, W3, b3):
    global _NC_CACHE
    if _NC_CACHE is None:
        _NC_CACHE = _build()
    nc = _NC_CACHE

    in_maps = make_in_maps(z1, z2, W1, b1, W2, b2, W3, b3)
    res = bass_utils.run_bass_kernel_spmd(nc, in_maps, core_ids=list(range(NCORES)))
    return np.concatenate([r["out"] for r in res.results], axis=0)


if __name__ == "__main__":
    rng = np.random.default_rng(0)
    s1 = 1.0 / np.sqrt(2 * EMB)
    s2 = 1.0 / np.sqrt(HID)
    ins = dict(
        z1=rng.standard_normal((N, EMB), dtype=np.float32),
        z2=rng.standard_normal((N, EMB), dtype=np.float32),
        W1=rng.uniform(-s1, s1, (HID, 2 * EMB)).astype(np.float32),
        b1=rng.uniform(-s1, s1, (HID,)).astype(np.float32),
        W2=rng.uniform(-s2, s2, (HID, HID)).astype(np.float32),
        b2=rng.uniform(-s2, s2, (HID,)).astype(np.float32),
        W3=rng.uniform(-s2, s2, (1, HID)).astype(np.float32),
        b3=rng.uniform(-s2, s2, (1,)).astype(np.float32),
    )
    out = kernel(**ins)
    print("out", out.shape, out.dtype, out[:2, :4])


# revision 47
# speedup vs baseline: 1.0005x; 1.0005x over previous
"""Fused pairwise-MLP kernel for Trainium2 (8 NeuronCores, SPMD data-parallel).

Computes log_q[i, j] = W3 @ gelu(W2 @ gelu(a[j] + b[i] + b1) + b2) + b3
with a = z1 @ W1a.T, b = z2 @ W1b.T  (W1 = [W1a | W1b]), N=1024, H=EMB=128.

Sharding: rows of i (z2) split across 8 cores, z1 + weights replicated
(host-side sharding; no collectives). The [N, N, H] intermediates are never
materialized in HBM — everything stays in SBUF/PSUM per 128-row i-tile.

The host only relays/relabels inputs (transposes, sharding, zero-padding W3);
all math runs on device. Per core and per i: one 1024-wide gelu on ACT
(bias port adds b[i]+b1), two fp16 W2 matmuls, a second gelu, and the W3
dot as fp16 M=32 column-tile matmuls batched 4-concurrent per quad of i's.
The kernel is ACT-bound: 2 gelu passes over 16.8M elements per core at
1 elem/lane/cycle @ 1.2 GHz.
"""

import numpy as np

import concourse.bacc as bacc
import concourse.bass as bass
import concourse.tile as tile
import concourse.mybir as mybir
from concourse import bass_utils



N = 1024
EMB = 128
HID = 128
NCORES = 8
SH = N // NCORES  # i-rows per core
F32 = mybir.dt.float32
FP16 = mybir.dt.float16
GELU = mybir.ActivationFunctionType.Gelu


def _build():
    nc = bacc.Bacc("TRN2", target_bir_lowering=False, debug=False)

    z1Tq_d = [
        nc.dram_tensor(f"z1Tq{q}", (EMB, 256), F32, kind="ExternalInput")
        for q in range(4)
    ]
    z2T_d = nc.dram_tensor("z2T", (EMB, SH), F32, kind="ExternalInput")
    w1aT_d = nc.dram_tensor("w1aT", (EMB, HID), F32, kind="ExternalInput")
    w1bT_d = nc.dram_tensor("w1bT", (EMB, HID), F32, kind="ExternalInput")
    w2T_d = nc.dram_tensor("w2T", (HID, HID), F32, kind="ExternalInput")
    w3p_d = nc.dram_tensor("w3p", (HID, 32), F32, kind="ExternalInput")
    b1_d = nc.dram_tensor("b1", (HID,), F32, kind="ExternalInput")
    b2_d = nc.dram_tensor("b2", (HID,), F32, kind="ExternalInput")
    b3_d = nc.dram_tensor("b3", (1,), F32, kind="ExternalInput")
    out_d = nc.dram_tensor("out", (SH, N), F32, kind="ExternalOutput")

    with tile.TileContext(nc) as tc:
        _body(
            tc, out_d, z1Tq_d, z2T_d, w1aT_d, w1bT_d, w2T_d, w3p_d,
            b1_d, b2_d, b3_d,
        )

    nc.compile()
    return nc


def _body(tc, out_d, z1Tq_d, z2T_d, w1aT_d, w1bT_d, w2T_d, w3p_d, b1_d, b2_d, b3_d):
    nc = tc.nc
    with (
        tc.tile_pool(name="const", bufs=1) as const,
        tc.tile_pool(name="h1p", bufs=4) as h1p,
        tc.tile_pool(name="h2p", bufs=8) as h2p,
        tc.tile_pool(name="srows", bufs=8) as srows,
        tc.tile_pool(name="ps", bufs=3, space="PSUM") as ps,
        tc.tile_pool(name="rowp", bufs=1, space="PSUM") as rowp,
    ):
        # ---- load inputs: z1T fans out across the sync+scalar HWDGE queues,
        # small tensors ride the gpsimd SWDGE queue ----
        w1aT_sb = const.tile([128, HID], F32)
        nc.sync.dma_start(out=w1aT_sb, in_=w1aT_d.ap())
        z1T_sb = const.tile([128, N], F32)
        for q, eng in enumerate((nc.sync, nc.scalar, nc.sync, nc.scalar)):
            eng.dma_start(out=z1T_sb[:, q * 256 : (q + 1) * 256], in_=z1Tq_d[q].ap())
        z2T_sb = const.tile([128, SH], F32)
        nc.sync.dma_start(out=z2T_sb, in_=z2T_d.ap())
        w1bT_sb = const.tile([128, HID], F32)
        nc.scalar.dma_start(out=w1bT_sb, in_=w1bT_d.ap())
        w2T_f = const.tile([128, HID], F32)
        nc.scalar.dma_start(out=w2T_f, in_=w2T_d.ap())
        w3p_f = const.tile([128, 32], F32)
        nc.gpsimd.dma_start(out=w3p_f, in_=w3p_d.ap())
        b1_sb = const.tile([128, 1], F32)
        nc.sync.dma_start(out=b1_sb, in_=b1_d.ap().rearrange("(p o) -> p o", o=1))
        b2_sb = const.tile([128, 1], F32)
        nc.gpsimd.dma_start(out=b2_sb, in_=b2_d.ap().rearrange("(p o) -> p o", o=1))
        b3_sb = const.tile([128, 1], F32)
        nc.gpsimd.dma_start(
            out=b3_sb,
            in_=bass.AP(tensor=b3_d, offset=0, ap=[[0, 128], [1, 1]]),
        )

        # Dummy 1-element gelu so the ~2.7us ACT table load for the gelu set
        # runs right away, off the critical path of the first real gelu.
        tiny = const.tile([1, 1], F32)
        nc.vector.memset(tiny, 0.0)
        warm = const.tile([1, 1], F32)
        nc.scalar.activation(warm, tiny, GELU)

        # fp32r / bf16 lhsT casts (DVE rounds on write, as the fp32r matmul
        # consumer requires)
        w2T_sb = const.tile([128, HID], FP16)
        nc.vector.tensor_copy(w2T_sb, w2T_f)
        w3T_sb = const.tile([128, 32], FP16)
        nc.vector.tensor_copy(w3T_sb, w3p_f)

        # ---- a[h, j] for all j (SBUF: DVE pre-adds read it); b_pp = b + b1 ----
        tpa = ps.tile([128, 1024], F32, tag="ps1")
        for q in range(4):
            nc.tensor.matmul(
                tpa[:, q * 256 : (q + 1) * 256],
                w1aT_sb,
                z1T_sb[:, q * 256 : (q + 1) * 256],
            )
        tpb = ps.tile([128, 1024], F32, tag="ps1")
        nc.tensor.matmul(tpb[:, 0:SH], w1bT_sb, z2T_sb)
        b_pp_sb = const.tile([128, SH], F32)
        nc.vector.tensor_scalar_add(b_pp_sb, tpb[:, 0:SH], b1_sb[:, 0:1])

        # a stays in PSUM for the head quads' gelus; the SBUF copy (for the
        # steady-state DVE pre-adds) happens off the critical path.
        a_sb = const.tile([128, N], F32)
        nc.vector.tensor_copy(a_sb, tpa)

        # ---- main loop over my 128 i values, processed as 32 quads ----
        # DVE precomputes pre[:, k] = a + b_pp[:, i] for the 4 i's of a quad
        # (2x-mode SBUF adds) so ACT runs one bias-free 4096-wide gelu1 per
        # quad plus one 1024-wide gelu2 per i, emitted 2 quads ahead. The
        # W3-dot runs as 4 concurrent fp16 M=32 column-tile matmuls (outputs
        # at PSUM partitions 0/32/64/96); one strided DVE op evacuates all 4
        # rows, and one DMA per quad writes them out.
        NQ = SH // 4
        h1qs = [None] * NQ

        def emit_g1_quad(q):
            h1q = h1p.tile([128, 4 * N], FP16, tag="h1q", name="h1q", bufs=3)
            if q < 2:
                # Head quads: per-i gelu with the ACT bias port, streaming a
                # straight from PSUM — skips the DVE pre-add chain and the
                # a->SBUF copy on the kernel's critical path.
                for k in range(4):
                    i = 4 * q + k
                    nc.scalar.activation(
                        h1q[:, k * N : (k + 1) * N],
                        tpa,
                        GELU,
                        bias=b_pp_sb[:, i : i + 1],
                    )
            else:
                pre = h1p.tile([128, 4 * N], F32, tag="pre", name="pre", bufs=3)
                for k in range(4):
                    i = 4 * q + k
                    nc.vector.tensor_scalar_add(
                        pre[:, k * N : (k + 1) * N], a_sb, b_pp_sb[:, i : i + 1]
                    )
                nc.scalar.activation(h1q, pre, GELU)
            h1qs[q] = h1q

        emit_g1_quad(0)
        emit_g1_quad(1)
        h2s = [None] * 4
        for q in range(NQ):
            if q + 2 < NQ:
                emit_g1_quad(q + 2)
            h1q = h1qs[q]
            h1qs[q] = None

            for k in range(4):
                base = k * N
                ps1 = ps.tile([128, N], F32, tag="ps1")
                nc.tensor.matmul(ps1[:, 0:512], w2T_sb, h1q[:, base : base + 512])
                nc.tensor.matmul(
                    ps1[:, 512:1024], w2T_sb, h1q[:, base + 512 : base + 1024]
                )
                h2 = h2p.tile([128, N], FP16, tag="h2")
                nc.scalar.activation(h2, ps1, GELU, bias=b2_sb[:, 0:1])
                h2s[k] = h2

            pr = rowp.tile([128, N], F32, tag="row", name="pr")
            for k in range(4):
                nc.tensor.matmul(
                    pr[32 * k : 32 * k + 32, 0:512],
                    w3T_sb,
                    h2s[k][:, 0:512],
                    tile_position=(0, 32 * k),
                )
                nc.tensor.matmul(
                    pr[32 * k : 32 * k + 32, 512:1024],
                    w3T_sb,
                    h2s[k][:, 512:1024],
                    tile_position=(0, 32 * k),
                )
            # One full-width DVE op evacuates all 4 result rows (at partitions
            # 0/32/64/96; cost is free-dim-bound, same as a single row); the
            # DMA gathers the 4 strided rows out.
            srow4 = srows.tile([128, N], F32, tag="srow", bufs=3)
            nc.vector.tensor_scalar_add(srow4, pr, b3_sb[:, 0:1])
            nc.sync.dma_start(
                out=out_d.ap()[4 * q : 4 * q + 4, :], in_=srow4[0:128:32, :]
            )


_NC_CACHE = None


def make_in_maps(z1, z2, W1, b1, W2, b2, W3, b3):
    f = np.float32
    z1 = np.asarray(z1, dtype=f)
    z2 = np.asarray(z2, dtype=f)
    W1 = np.asarray(W1, dtype=f)
    b1 = np.ascontiguousarray(np.asarray(b1, dtype=f))
    W2 = np.asarray(W2, dtype=f)
    b2 = np.ascontiguousarray(np.asarray(b2, dtype=f))
    W3 = np.asarray(W3, dtype=f)
    b3 = np.ascontiguousarray(np.asarray(b3, dtype=f))

    # Host-side relayout only (no math): transposes, the i-shard split of
    # z2, and zero-padding W3 to an M=32 column tile.
    z1T = np.ascontiguousarray(z1.T)
    z1Tq = {
        f"z1Tq{q}": np.ascontiguousarray(z1T[:, q * 256 : (q + 1) * 256])
        for q in range(4)
    }
    w1aT = np.ascontiguousarray(W1[:, :EMB].T)
    w1bT = np.ascontiguousarray(W1[:, EMB:].T)
    w2T = np.ascontiguousarray(W2.T)
    w3p = np.zeros((HID, 32), dtype=f)
    w3p[:, 0] = W3[0]

    return [
        {
            **z1Tq,
            "z2T": np.ascontiguousarray(z2[c * SH : (c + 1) * SH].T),
            "w1aT": w1aT,
            "w1bT": w1bT,
            "w2T": w2T,
            "w3p": w3p,
            "b1": b1,
            "b2": b2,
            "b3": b3,
        }
        for c in range(NCORES)
    ]


def kernel(z1, z2, W1, b1, W2, b2, W3, b3):
    global _NC_CACHE
    if _NC_CACHE is None:
        _NC_CACHE = _build()
    nc = _NC_CACHE

    in_maps = make_in_maps(z1, z2, W1, b1, W2, b2, W3, b3)
    res = bass_utils.run_bass_kernel_spmd(nc, in_maps, core_ids=list(range(NCORES)))
    return np.concatenate([r["out"] for r in res.results], axis=0)


if __name__ == "__main__":
    rng = np.random.default_rng(0)
    s1 = 1.0 / np.sqrt(2 * EMB)
    s2 = 1.0 / np.sqrt(HID)
    ins = dict(
        z1=rng.standard_normal((N, EMB), dtype=np.float32),
        z2=rng.standard_normal((N, EMB), dtype=np.float32),
        W1=rng.uniform(-s1, s1, (HID, 2 * EMB)).astype(np.float32),
        b1=rng.uniform(-s1, s1, (HID,)).astype(np.float32),
        W2=rng.uniform(-s2, s2, (HID, HID)).astype(np.float32),
        b2=rng.uniform(-s2, s2, (HID,)).astype(np.float32),
        W3=rng.uniform(-s2, s2, (1, HID)).astype(np.float32),
        b3=rng.uniform(-s2, s2, (1,)).astype(np.float32),
    )
    out = kernel(**ins)
    print("out", out.shape, out.dtype, out[:2, :4])


# revision 48
# speedup vs baseline: 1.0034x; 1.0029x over previous
"""Fused pairwise-MLP kernel for Trainium2 (8 NeuronCores, SPMD data-parallel).

Computes log_q[i, j] = W3 @ gelu(W2 @ gelu(a[j] + b[i] + b1) + b2) + b3
with a = z1 @ W1a.T, b = z2 @ W1b.T  (W1 = [W1a | W1b]), N=1024, H=EMB=128.

Sharding: rows of i (z2) split across 8 cores, z1 + weights replicated
(host-side sharding; no collectives). The [N, N, H] intermediates are never
materialized in HBM — everything stays in SBUF/PSUM per 128-row i-tile.

The host only relays/relabels inputs (transposes, sharding, zero-padding W3);
all math runs on device. Per core and per i: one 1024-wide gelu on ACT
(bias port adds b[i]+b1), two fp16 W2 matmuls, a second gelu, and the W3
dot as fp16 M=32 column-tile matmuls batched 4-concurrent per quad of i's.
The kernel is ACT-bound: 2 gelu passes over 16.8M elements per core at
1 elem/lane/cycle @ 1.2 GHz.
"""

import numpy as np

import concourse.bacc as bacc
import concourse.bass as bass
import concourse.tile as tile
import concourse.mybir as mybir
from concourse import bass_utils



N = 1024
EMB = 128
HID = 128
NCORES = 8
SH = N // NCORES  # i-rows per core
F32 = mybir.dt.float32
FP16 = mybir.dt.float16
GELU = mybir.ActivationFunctionType.Gelu


def _build():
    nc = bacc.Bacc("TRN2", target_bir_lowering=False, debug=False)

    z1Tq_d = [
        nc.dram_tensor(f"z1Tq{q}", (EMB, 256), F32, kind="ExternalInput")
        for q in range(4)
    ]
    z2T_d = nc.dram_tensor("z2T", (EMB, SH), F32, kind="ExternalInput")
    w1aT_d = nc.dram_tensor("w1aT", (EMB, HID), F32, kind="ExternalInput")
    w1bT_d = nc.dram_tensor("w1bT", (EMB, HID), F32, kind="ExternalInput")
    w2T_d = nc.dram_tensor("w2T", (HID, HID), F32, kind="ExternalInput")
    w3p_d = nc.dram_tensor("w3p", (HID, 32), F32, kind="ExternalInput")
    b1_d = nc.dram_tensor("b1", (HID,), F32, kind="ExternalInput")
    b2_d = nc.dram_tensor("b2", (HID,), F32, kind="ExternalInput")
    b3_d = nc.dram_tensor("b3", (1,), F32, kind="ExternalInput")
    out_d = nc.dram_tensor("out", (SH, N), F32, kind="ExternalOutput")

    with tile.TileContext(nc) as tc:
        _body(
            tc, out_d, z1Tq_d, z2T_d, w1aT_d, w1bT_d, w2T_d, w3p_d,
            b1_d, b2_d, b3_d,
        )

    nc.compile()
    return nc


def _body(tc, out_d, z1Tq_d, z2T_d, w1aT_d, w1bT_d, w2T_d, w3p_d, b1_d, b2_d, b3_d):
    nc = tc.nc
    with (
        tc.tile_pool(name="const", bufs=1) as const,
        tc.tile_pool(name="h1p", bufs=4) as h1p,
        tc.tile_pool(name="h2p", bufs=8) as h2p,
        tc.tile_pool(name="srows", bufs=8) as srows,
        tc.tile_pool(name="ps", bufs=3, space="PSUM") as ps,
        tc.tile_pool(name="rowp", bufs=1, space="PSUM") as rowp,
    ):
        # ---- load inputs: z1T fans out across the sync+scalar HWDGE queues,
        # small tensors ride the gpsimd SWDGE queue ----
        w1aT_sb = const.tile([128, HID], F32)
        nc.sync.dma_start(out=w1aT_sb, in_=w1aT_d.ap())
        z1T_sb = const.tile([128, N], F32)
        for q, eng in enumerate((nc.sync, nc.scalar, nc.sync, nc.scalar)):
            eng.dma_start(out=z1T_sb[:, q * 256 : (q + 1) * 256], in_=z1Tq_d[q].ap())
        z2T_sb = const.tile([128, SH], F32)
        nc.sync.dma_start(out=z2T_sb, in_=z2T_d.ap())
        w1bT_sb = const.tile([128, HID], F32)
        nc.scalar.dma_start(out=w1bT_sb, in_=w1bT_d.ap())
        w2T_f = const.tile([128, HID], F32)
        nc.scalar.dma_start(out=w2T_f, in_=w2T_d.ap())
        w3p_f = const.tile([128, 32], F32)
        nc.gpsimd.dma_start(out=w3p_f, in_=w3p_d.ap())
        b1_sb = const.tile([128, 1], F32)
        nc.sync.dma_start(out=b1_sb, in_=b1_d.ap().rearrange("(p o) -> p o", o=1))
        b2_sb = const.tile([128, 1], F32)
        nc.gpsimd.dma_start(out=b2_sb, in_=b2_d.ap().rearrange("(p o) -> p o", o=1))
        b3_sb = const.tile([128, 1], F32)
        nc.gpsimd.dma_start(
            out=b3_sb,
            in_=bass.AP(tensor=b3_d, offset=0, ap=[[0, 128], [1, 1]]),
        )

        # Dummy 1-element gelu so the ~2.7us ACT table load for the gelu set
        # runs right away, off the critical path of the first real gelu.
        tiny = const.tile([1, 1], F32)
        nc.vector.memset(tiny, 0.0)
        warm = const.tile([1, 1], F32)
        nc.scalar.activation(warm, tiny, GELU)

        # fp16 lhsT casts (DVE rounds on write)
        w2T_sb = const.tile([128, HID], FP16)
        nc.vector.tensor_copy(w2T_sb, w2T_f)
        w3T_sb = const.tile([128, 32], FP16)
        nc.vector.tensor_copy(w3T_sb, w3p_f)

        # ---- a[h, j] for all j (SBUF: DVE pre-adds read it); b_pp = b + b1 ----
        tpa = ps.tile([128, 1024], F32, tag="ps1")
        for q in range(4):
            nc.tensor.matmul(
                tpa[:, q * 256 : (q + 1) * 256],
                w1aT_sb,
                z1T_sb[:, q * 256 : (q + 1) * 256],
            )
        tpb = ps.tile([128, 1024], F32, tag="ps1")
        nc.tensor.matmul(tpb[:, 0:SH], w1bT_sb, z2T_sb)
        b_pp_sb = const.tile([128, SH], F32)
        nc.vector.tensor_scalar_add(b_pp_sb, tpb[:, 0:SH], b1_sb[:, 0:1])

        # a stays in PSUM for the head quads' gelus; the SBUF copy (for the
        # steady-state DVE pre-adds) happens off the critical path.
        a_sb = const.tile([128, N], F32)
        nc.vector.tensor_copy(a_sb, tpa)

        # ---- main loop over my 128 i values, processed as 32 quads ----
        # DVE precomputes pre[:, k] = a + b_pp[:, i] for the 4 i's of a quad
        # (2x-mode SBUF adds) so ACT runs one bias-free 4096-wide gelu1 per
        # quad plus one 1024-wide gelu2 per i, emitted 2 quads ahead. The
        # W3-dot runs as 4 concurrent fp16 M=32 column-tile matmuls (outputs
        # at PSUM partitions 0/32/64/96); one strided DVE op evacuates all 4
        # rows, and one DMA per quad writes them out.
        NQ = SH // 4
        h1qs = [None] * NQ

        def emit_g1_quad(q):
            h1q = h1p.tile([128, 4 * N], FP16, tag="h1q", name="h1q", bufs=3)
            if q < 2:
                # Head quads: per-i gelu with the ACT bias port, streaming a
                # straight from PSUM — skips the DVE pre-add chain and the
                # a->SBUF copy on the kernel's critical path.
                for k in range(4):
                    i = 4 * q + k
                    nc.scalar.activation(
                        h1q[:, k * N : (k + 1) * N],
                        tpa,
                        GELU,
                        bias=b_pp_sb[:, i : i + 1],
                    )
            else:
                pre = h1p.tile([128, 4 * N], F32, tag="pre", name="pre", bufs=3)
                for k in range(4):
                    i = 4 * q + k
                    nc.vector.tensor_scalar_add(
                        pre[:, k * N : (k + 1) * N], a_sb, b_pp_sb[:, i : i + 1]
                    )
                nc.scalar.activation(h1q, pre, GELU)
            h1qs[q] = h1q

        emit_g1_quad(0)
        emit_g1_quad(1)
        h2s = [None] * 4
        for q in range(NQ):
            if q + 2 < NQ:
                emit_g1_quad(q + 2)
            h1q = h1qs[q]
            h1qs[q] = None

            for k in range(4):
                base = k * N
                ps1 = ps.tile([128, N], F32, tag="ps1")
                nc.tensor.matmul(ps1[:, 0:512], w2T_sb, h1q[:, base : base + 512])
                nc.tensor.matmul(
                    ps1[:, 512:1024], w2T_sb, h1q[:, base + 512 : base + 1024]
                )
                h2 = h2p.tile([128, N], FP16, tag="h2")
                nc.scalar.activation(h2, ps1, GELU, bias=b2_sb[:, 0:1])
                h2s[k] = h2

            pr = rowp.tile([128, N], F32, tag="row", name="pr")
            for k in range(4):
                nc.tensor.matmul(
                    pr[32 * k : 32 * k + 32, 0:512],
                    w3T_sb,
                    h2s[k][:, 0:512],
                    tile_position=(0, 32 * k),
                )
                nc.tensor.matmul(
                    pr[32 * k : 32 * k + 32, 512:1024],
                    w3T_sb,
                    h2s[k][:, 512:1024],
                    tile_position=(0, 32 * k),
                )
            # One full-width DVE op evacuates all 4 result rows (at partitions
            # 0/32/64/96; cost is free-dim-bound, same as a single row); the
            # DMA gathers the 4 strided rows out.
            srow4 = srows.tile([128, N], F32, tag="srow", bufs=3)
            nc.vector.tensor_scalar_add(srow4, pr, b3_sb[:, 0:1])
            nc.sync.dma_start(
                out=out_d.ap()[4 * q : 4 * q + 4, :], in_=srow4[0:128:32, :]
            )


_NC_CACHE = None


def make_in_maps(z1, z2, W1, b1, W2, b2, W3, b3):
    f = np.float32
    z1 = np.asarray(z1, dtype=f)
    z2 = np.asarray(z2, dtype=f)
    W1 = np.asarray(W1, dtype=f)
    b1 = np.ascontiguousarray(np.asarray(b1, dtype=f))
    W2 = np.asarray(W2, dtype=f)
    b2 = np.ascontiguousarray(np.asarray(b2, dtype=f))
    W3 = np.asarray(W3, dtype=f)
    b3 = np.ascontiguousarray(np.asarray(b3, dtype=f))

    # Host-side relayout only (no math): transposes, the i-shard split of
    # z2, and zero-padding W3 to an M=32 column tile.
    z1T = np.ascontiguousarray(z1.T)
    z1Tq = {
        f"z1Tq{q}": np.ascontiguousarray(z1T[:, q * 256 : (q + 1) * 256])
        for q in range(4)
    }
    w1aT = np.ascontiguousarray(W1[:, :EMB].T)
    w1bT = np.ascontiguousarray(W1[:, EMB:].T)
    w2T = np.ascontiguousarray(W2.T)
    w3p = np.zeros((HID, 32), dtype=f)
    w3p[:, 0] = W3[0]

    return [
        {
            **z1Tq,
            "z2T": np.ascontiguousarray(z2[c * SH : (c + 1) * SH].T),
            "w1aT": w1aT,
            "w1bT": w1bT,
            "w2T": w2T,
            "w3p": w3p,
            "b1": b1,
            "b2": b2,
            "b3": b3,
        }
        for c in range(NCORES)
    ]


def kernel(z1, z2, W1, b1, W2, b2, W3, b3):
    global _NC_CACHE
    if _NC_CACHE is None:
        _NC_CACHE = _build()
    nc = _NC_CACHE

    in_maps = make_in_maps(z1, z2, W1, b1, W2, b2, W3, b3)
    res = bass_utils.run_bass_kernel_spmd(nc, in_maps, core_ids=list(range(NCORES)))
    return np.concatenate([r["out"] for r in res.results], axis=0)


if __name__ == "__main__":
    rng = np.random.default_rng(0)
    s1 = 1.0 / np.sqrt(2 * EMB)
    s2 = 1.0 / np.sqrt(HID)
    ins = dict(
        z1=rng.standard_normal((N, EMB), dtype=np.float32),
        z2=rng.standard_normal((N, EMB), dtype=np.float32),
        W1=rng.uniform(-s1, s1, (HID, 2 * EMB)).astype(np.float32),
        b1=rng.uniform(-s1, s1, (HID,)).astype(np.float32),
        W2=rng.uniform(-s2, s2, (HID, HID)).astype(np.float32),
        b2=rng.uniform(-s2, s2, (HID,)).astype(np.float32),
        W3=rng.uniform(-s2, s2, (1, HID)).astype(np.float32),
        b3=rng.uniform(-s2, s2, (1,)).astype(np.float32),
    )
    out = kernel(**ins)
    print("out", out.shape, out.dtype, out[:2, :4])
